# revision 1
# baseline (speedup 1.0000x reference)
# MoE layer (top-2 of 8 experts, D=1024, H=4096) on 8 trn2 NeuronCores.
#
# Strategy (expert-parallel, per the sharding hint):
#   - Host computes the gate (x @ gate_w: 67 MFLOP, 0.05% of total work),
#     top-2 routing, softmax weights, expert_usage, top_i.
#   - Tokens are gathered per expert on host ("all-to-all by top-k
#     assignment" done during input sharding), padded to a common capacity C
#     so one SPMD NEFF serves all 8 cores. Core e gets expert e's weights
#     plus its routed tokens pre-transposed to [D, C].
#   - Device (per core): hT = relu(W1.T-tiles @ xT + b1) kept in SBUF,
#     o = (h @ W2) * gate_weight  -- both GEMMs as 128x128x512 matmuls in
#     bf16 with fp32 PSUM accumulation. Weights stay SBUF-resident; only
#     x-tiles and outputs stream.
#   - Host scatter-adds the (already gate-scaled) per-expert outputs back
#     to [T, DO] and adds the weights@b2 term.
import numpy as np
import ml_dtypes

import concourse.bacc as bacc
import concourse.mybir as mybir
import concourse.tile as tile
from concourse.bass import ts
from concourse.bass_utils import run_bass_kernel_spmd

BF16 = mybir.dt.bfloat16
F32 = mybir.dt.float32
bf16 = ml_dtypes.bfloat16

B, S, D, H, DO, E, K = 4, 2048, 1024, 4096, 1024, 8, 2
T = B * S
N_CORES = 8

_kernel_cache: dict[int, object] = {}


def _build_expert_kernel(C: int):
    """One-expert FFN kernel: o[C, DO] = wv * relu(x[C, D] @ W1 + b1) @ W2.

    C must be a multiple of 512. Token capacity per core.
    """
    assert C % 512 == 0
    MT = C // 128  # output m-subtiles (128 tokens each)
    TT = C // 512  # token tiles (512 tokens each)

    nc = bacc.Bacc(None, target_bir_lowering=False)

    # DRAM I/O. Layouts chosen so every DMA is partition-major contiguous.
    #   xT_d[tt, di, dd, tj]   = x[token tt*512+tj, d=dd*128+di]      (bf16)
    #   w1_d[di, hh, dd, hi]   = W1[dd*128+di, hh*128+hi]             (bf16)
    #   w2_d[hi, hh, do]       = W2[hh*128+hi, do]                    (bf16)
    #   b1_d[hi, hh]           = b1[hh*128+hi]                        (f32)
    #   wv_d[ti, mi]           = gate_weight[token mi*128+ti]         (f32)
    #   o_d[mi, ti, do]        = out[token mi*128+ti, do]             (f32)
    xT_d = nc.dram_tensor("xT", [TT, 128, 8, 512], BF16, kind="ExternalInput")
    w1_d = nc.dram_tensor("w1t", [128, 32, 8, 128], BF16, kind="ExternalInput")
    w2_d = nc.dram_tensor("w2t", [128, 32, 1024], BF16, kind="ExternalInput")
    b1_d = nc.dram_tensor("b1t", [128, 32], F32, kind="ExternalInput")
    wv_d = nc.dram_tensor("wv", [128, MT], F32, kind="ExternalInput")
    o_d = nc.dram_tensor("o", [MT, 128, 1024], F32, kind="ExternalOutput")

    relu = mybir.ActivationFunctionType.Relu

    with tile.TileContext(nc) as tc:
        with (
            tc.tile_pool(name="const", bufs=1) as const,
            tc.tile_pool(name="wres", bufs=1) as wres,
            tc.tile_pool(name="xp", bufs=2) as xp,
            tc.tile_pool(name="hp", bufs=1) as hp,
            tc.tile_pool(name="op", bufs=4) as op,
            tc.tile_pool(name="psA", bufs=4, space="PSUM") as psA,
            tc.tile_pool(name="psB", bufs=4, space="PSUM") as psB,
        ):
            b1_sb = const.tile([128, 32], F32)
            nc.sync.dma_start(b1_sb[:], b1_d[:])
            wv_sb = const.tile([128, MT], F32)
            nc.sync.dma_start(wv_sb[:], wv_d[:])

            # Expert weights, SBUF-resident for the whole kernel
            # (64KB + 64KB per partition in bf16).
            w1_sb = wres.tile([128, 32, 8, 128], BF16, tag="w1")
            w2_sb = wres.tile([128, 32, 1024], BF16, tag="w2")
            for q in range(4):
                nc.sync.dma_start(w1_sb[:, ts(q, 8), :, :], w1_d[:, ts(q, 8), :, :])
                nc.sync.dma_start(w2_sb[:, ts(q, 8), :], w2_d[:, ts(q, 8), :])

            for tt in range(TT):
                xc = xp.tile([128, 8, 512], BF16)
                nc.sync.dma_start(xc[:], xT_d[tt])

                # Stage A: hT[h, tok] = relu(x @ W1 + b1), h-major in SBUF.
                hs = hp.tile([128, 32, 512], BF16)
                for hh in range(32):
                    ps = psA.tile([128, 512], F32)
                    for dd in range(8):
                        nc.tensor.matmul(
                            ps[:],
                            w1_sb[:, hh, dd, :],
                            xc[:, dd, :],
                            start=(dd == 0),
                            stop=(dd == 7),
                        )
                    nc.scalar.activation(
                        hs[:, hh, :], ps[:], relu, bias=b1_sb[:, hh : hh + 1]
                    )

                # Stage B: o[tok, do] = (h @ W2) * wv, token-major.
                for mj in range(4):
                    mi = tt * 4 + mj
                    o_sb = op.tile([128, 1024], F32)
                    for do_t in range(2):
                        ps2 = psB.tile([128, 512], F32)
                        for hh in range(32):
                            nc.tensor.matmul(
                                ps2[:],
                                hs[:, hh, ts(mj, 128)],
                                w2_sb[:, hh, ts(do_t, 512)],
                                start=(hh == 0),
                                stop=(hh == 31),
                            )
                        nc.vector.tensor_scalar_mul(
                            o_sb[:, ts(do_t, 512)], ps2[:], wv_sb[:, mi : mi + 1]
                        )
                    nc.sync.dma_start(o_d[mi], o_sb[:])

    nc.compile()
    return nc


def _route(xf, gate_w, gate_b):
    """Host gating: replicates jax.lax.top_k(k=2) + softmax semantics."""
    logits = xf @ gate_w + gate_b  # [T, E] f32
    i1 = np.argmax(logits, axis=1)
    ar = np.arange(T)
    v1 = logits[ar, i1]
    masked = logits.copy()
    masked[ar, i1] = -np.inf
    i2 = np.argmax(masked, axis=1)
    v2 = logits[ar, i2]
    # softmax over the top-2 scores
    e2 = np.exp(v2 - v1)
    w1_ = 1.0 / (1.0 + e2)
    w2_ = e2 / (1.0 + e2)
    top_i = np.stack([i1, i2], axis=1).astype(np.int32)
    weights = np.zeros((T, E), np.float32)
    weights[ar, i1] = w1_
    weights[ar, i2] = w2_
    expert_usage = np.bincount(top_i.ravel(), minlength=E).astype(np.int32)
    return logits, top_i, weights, expert_usage


def kernel(x, gate_w, gate_b, w1, b1, w2, b2):
    x = np.asarray(x, np.float32)
    gate_w = np.asarray(gate_w, np.float32)
    gate_b = np.asarray(gate_b, np.float32)
    w1 = np.asarray(w1, np.float32)
    b1 = np.asarray(b1, np.float32)
    w2 = np.asarray(w2, np.float32)
    b2 = np.asarray(b2, np.float32)

    xf = x.reshape(T, D)
    _, top_i, weights, expert_usage = _route(xf, gate_w, gate_b)

    # Token lists per expert ("all-to-all" on host).
    idxs = [np.where((top_i == e).any(axis=1))[0] for e in range(E)]
    cnts = [len(ix) for ix in idxs]
    C = max(512, -(-max(cnts) // 512) * 512)
    MT = C // 128
    TT = C // 512

    if C not in _kernel_cache:
        _kernel_cache[C] = _build_expert_kernel(C)
    nc = _kernel_cache[C]

    in_maps = []
    for e in range(E):
        ix = idxs[e]
        cnt = cnts[e]
        xg = np.zeros((C, D), np.float32)
        xg[:cnt] = xf[ix]
        # xT_d[tt, di, dd, tj] = xg[tt*512+tj, dd*128+di]
        xT_d = np.ascontiguousarray(
            xg.reshape(TT, 512, 8, 128).transpose(0, 3, 2, 1).astype(bf16)
        )
        # w1_d[di, hh, dd, hi] = W1[dd*128+di, hh*128+hi]
        w1_d = np.ascontiguousarray(
            w1[e].reshape(8, 128, 32, 128).transpose(1, 2, 0, 3).astype(bf16)
        )
        # w2_d[hi, hh, do] = W2[hh*128+hi, do]
        w2_d = np.ascontiguousarray(
            w2[e].reshape(32, 128, DO).transpose(1, 0, 2).astype(bf16)
        )
        b1_d = np.ascontiguousarray(b1[e].reshape(32, 128).T.astype(np.float32))
        wv = np.zeros((C,), np.float32)
        wv[:cnt] = weights[ix, e]
        wv_d = np.ascontiguousarray(wv.reshape(MT, 128).T)
        in_maps.append(
            {"xT": xT_d, "w1t": w1_d, "w2t": w2_d, "b1t": b1_d, "wv": wv_d}
        )

    res = run_bass_kernel_spmd(nc, in_maps, core_ids=list(range(N_CORES)))

    out = np.zeros((T, DO), np.float32)
    for e in range(E):
        o_e = res.results[e]["o"].reshape(C, DO)
        out[idxs[e]] += o_e[: cnts[e]]
    out += weights @ b2  # b2 term, folded with the gate weights

    return out.reshape(B, S, DO), weights, expert_usage, top_i


# revision 6
# speedup vs baseline: 1.1923x; 1.1923x over previous
# MoE layer (top-2 of 8 experts, D=1024, H=4096) on 8 trn2 NeuronCores.
#
# Strategy (expert-parallel, per the sharding hint):
#   - Host computes the gate (x @ gate_w: 67 MFLOP, 0.05% of total work),
#     top-2 routing, softmax weights, expert_usage, top_i.
#   - Tokens are gathered per expert on host ("all-to-all by top-k
#     assignment" done during input sharding), padded to a common capacity C
#     so one SPMD NEFF serves all 8 cores. Core e gets expert e's weights
#     plus its routed tokens pre-transposed to [D, C].
#   - Device (per core): hT = relu(W1.T-tiles @ xT + b1) kept in SBUF,
#     o = (h @ W2) * gate_weight  -- both GEMMs as 128x128x512 matmuls in
#     bf16 with fp32 PSUM accumulation. Weights stay SBUF-resident; only
#     x-tiles and outputs stream.
#   - Host scatter-adds the (already gate-scaled) per-expert outputs back
#     to [T, DO] and adds the weights@b2 term.
import numpy as np
import ml_dtypes

import concourse.bacc as bacc
import concourse.mybir as mybir
import concourse.tile as tile
from concourse.bass import ts
from concourse.bass_utils import run_bass_kernel_spmd

BF16 = mybir.dt.bfloat16
F32 = mybir.dt.float32
bf16 = ml_dtypes.bfloat16

B, S, D, H, DO, E, K = 4, 2048, 1024, 4096, 1024, 8, 2
T = B * S
N_CORES = 8

_kernel_cache: dict[int, object] = {}


def _build_expert_kernel(C: int):
    """One-expert FFN kernel: o[C, DO] = wv * relu(x[C, D] @ W1 + b1) @ W2.

    C must be a multiple of 128. Token capacity per core. The last token
    tile may be partial (N in {128, 256, 384}).
    """
    assert C % 128 == 0
    MT = C // 128  # output m-subtiles (128 tokens each)
    TT = -(-C // 512)  # token tiles (<=512 tokens each)
    CP = TT * 512  # DRAM-padded capacity (host pads x to this)

    nc = bacc.Bacc(None, target_bir_lowering=False)

    # DRAM I/O. Layouts chosen so every DMA is partition-major contiguous.
    #   xT_d[tt, di, dd, tj]   = x[token tt*512+tj, d=dd*128+di]      (bf16)
    #   w1_d[di, hh, dd, hi]   = W1[dd*128+di, hh*128+hi]             (bf16)
    #   w2_d[hi, hh, do]       = W2[hh*128+hi, do]                    (bf16)
    #   b1_d[hi, hh]           = b1[hh*128+hi]                        (f32)
    #   wv_d[ti, mi]           = gate_weight[token mi*128+ti]         (f32)
    #   o_d[mi, ti, do]        = out[token mi*128+ti, do]             (f32)
    del CP  # host pads x to TT*512; device reads only the first C tokens
    xT_d = nc.dram_tensor("xT", [TT, 128, 8, 512], BF16, kind="ExternalInput")
    w1_d = nc.dram_tensor("w1t", [128, 32, 8, 128], BF16, kind="ExternalInput")
    w2_d = nc.dram_tensor("w2t", [128, 32, 1024], BF16, kind="ExternalInput")
    b1_d = nc.dram_tensor("b1t", [128, 32], F32, kind="ExternalInput")
    wv_d = nc.dram_tensor("wv", [128, MT], F32, kind="ExternalInput")
    o_d = nc.dram_tensor("o", [MT, 128, 1024], F32, kind="ExternalOutput")

    relu = mybir.ActivationFunctionType.Relu

    with tile.TileContext(nc) as tc:
        with (
            tc.tile_pool(name="const", bufs=1) as const,
            tc.tile_pool(name="wres", bufs=1) as wres,
            tc.tile_pool(name="xp", bufs=2) as xp,
            tc.tile_pool(name="hp", bufs=1) as hp,
            tc.tile_pool(name="op", bufs=4) as op,
            tc.tile_pool(name="psA", bufs=4, space="PSUM") as psA,
            tc.tile_pool(name="psB", bufs=4, space="PSUM") as psB,
        ):
            # DMA issue order matters for time-to-first-matmul: the first
            # stage-A matmul needs only x(tt=0) and the first w1 chunk, so
            # those go first; w2 (not read until stage B, ~55us later) last.
            xc0 = xp.tile([128, 8, 512], BF16, tag="xc")
            nc.sync.dma_start(xc0[:], xT_d[0])
            b1_sb = const.tile([128, 32], F32)
            nc.sync.dma_start(b1_sb[:], b1_d[:])
            wv_sb = const.tile([128, MT], F32)
            nc.sync.dma_start(wv_sb[:], wv_d[:])

            # Expert weights, SBUF-resident for the whole kernel
            # (64KB + 64KB per partition in bf16), in 4-hh chunk tiles so
            # stage A can start as soon as its first chunk lands.
            w1_sb = [
                wres.tile([128, 4, 8, 128], BF16, tag=f"w1q{q}", name=f"w1q{q}")
                for q in range(8)
            ]
            w2_sb = [
                wres.tile([128, 8, 1024], BF16, tag=f"w2q{q}", name=f"w2q{q}")
                for q in range(4)
            ]
            for q in range(8):
                nc.sync.dma_start(w1_sb[q][:], w1_d[:, ts(q, 4), :, :])
            for q in range(4):
                nc.sync.dma_start(w2_sb[q][:], w2_d[:, ts(q, 8), :])

            for tt in range(TT):
                ntok = min(512, C - tt * 512)
                if tt == 0:
                    xc = xc0
                else:
                    xc = xp.tile([128, 8, 512], BF16, tag="xc")
                    nc.sync.dma_start(
                        xc[:, :, :ntok], xT_d[tt][:, :, :ntok]
                    )

                # Stage A: hT[h, tok] = relu(x @ W1 + b1), h-major in SBUF.
                hs = hp.tile([128, 32, 512], BF16)
                for hh in range(32):
                    ps = psA.tile([128, 512], F32)
                    for dd in range(8):
                        nc.tensor.matmul(
                            ps[:, :ntok],
                            w1_sb[hh // 4][:, hh % 4, dd, :],
                            xc[:, dd, :ntok],
                            start=(dd == 0),
                            stop=(dd == 7),
                        )
                    nc.scalar.activation(
                        hs[:, hh, :ntok], ps[:, :ntok], relu, bias=b1_sb[:, hh : hh + 1]
                    )

                # Stage B: o[tok, do] = (h @ W2) * wv, token-major.
                for mj in range(ntok // 128):
                    mi = tt * 4 + mj
                    o_sb = op.tile([128, 1024], F32)
                    for do_t in range(2):
                        ps2 = psB.tile([128, 512], F32)
                        for hh in range(32):
                            nc.tensor.matmul(
                                ps2[:],
                                hs[:, hh, ts(mj, 128)],
                                w2_sb[hh // 8][:, hh % 8, ts(do_t, 512)],
                                start=(hh == 0),
                                stop=(hh == 31),
                            )
                        nc.vector.tensor_scalar_mul(
                            o_sb[:, ts(do_t, 512)], ps2[:], wv_sb[:, mi : mi + 1]
                        )
                    nc.sync.dma_start(o_d[mi], o_sb[:])

    nc.compile()
    return nc


def _route(xf, gate_w, gate_b):
    """Host gating: replicates jax.lax.top_k(k=2) + softmax semantics."""
    logits = xf @ gate_w + gate_b  # [T, E] f32
    i1 = np.argmax(logits, axis=1)
    ar = np.arange(T)
    v1 = logits[ar, i1]
    masked = logits.copy()
    masked[ar, i1] = -np.inf
    i2 = np.argmax(masked, axis=1)
    v2 = logits[ar, i2]
    # softmax over the top-2 scores
    e2 = np.exp(v2 - v1)
    w1_ = 1.0 / (1.0 + e2)
    w2_ = e2 / (1.0 + e2)
    top_i = np.stack([i1, i2], axis=1).astype(np.int32)
    weights = np.zeros((T, E), np.float32)
    weights[ar, i1] = w1_
    weights[ar, i2] = w2_
    expert_usage = np.bincount(top_i.ravel(), minlength=E).astype(np.int32)
    return logits, top_i, weights, expert_usage


def _plan(xf, gate_w, gate_b):
    """Routing + capacity plan. Returns (top_i, weights, expert_usage,
    idxs, cnts, C)."""
    _, top_i, weights, expert_usage = _route(xf, gate_w, gate_b)
    idxs = [np.where((top_i == e).any(axis=1))[0] for e in range(E)]
    cnts = [len(ix) for ix in idxs]
    C = max(512, -(-max(cnts) // 128) * 128)
    return top_i, weights, expert_usage, idxs, cnts, C


def _make_in_maps(xf, w1, b1, w2, weights, idxs, cnts, C):
    MT = C // 128
    TT = -(-C // 512)
    CP = TT * 512
    in_maps = []
    for e in range(E):
        ix = idxs[e]
        cnt = cnts[e]
        xg = np.zeros((CP, D), np.float32)
        xg[:cnt] = xf[ix]
        # xT_d[tt, di, dd, tj] = xg[tt*512+tj, dd*128+di]
        xT_d = np.ascontiguousarray(
            xg.reshape(TT, 512, 8, 128).transpose(0, 3, 2, 1).astype(bf16)
        )
        # w1_d[di, hh, dd, hi] = W1[dd*128+di, hh*128+hi]
        w1_d = np.ascontiguousarray(
            w1[e].reshape(8, 128, 32, 128).transpose(1, 2, 0, 3).astype(bf16)
        )
        # w2_d[hi, hh, do] = W2[hh*128+hi, do]
        w2_d = np.ascontiguousarray(
            w2[e].reshape(32, 128, DO).transpose(1, 0, 2).astype(bf16)
        )
        b1_d = np.ascontiguousarray(b1[e].reshape(32, 128).T.astype(np.float32))
        wv = np.zeros((C,), np.float32)
        wv[:cnt] = weights[ix, e]
        wv_d = np.ascontiguousarray(wv.reshape(MT, 128).T)
        in_maps.append(
            {"xT": xT_d, "w1t": w1_d, "w2t": w2_d, "b1t": b1_d, "wv": wv_d}
        )
    return in_maps


def kernel(x, gate_w, gate_b, w1, b1, w2, b2):
    x = np.asarray(x, np.float32)
    gate_w = np.asarray(gate_w, np.float32)
    gate_b = np.asarray(gate_b, np.float32)
    w1 = np.asarray(w1, np.float32)
    b1 = np.asarray(b1, np.float32)
    w2 = np.asarray(w2, np.float32)
    b2 = np.asarray(b2, np.float32)

    xf = x.reshape(T, D)
    top_i, weights, expert_usage, idxs, cnts, C = _plan(xf, gate_w, gate_b)

    if C not in _kernel_cache:
        _kernel_cache[C] = _build_expert_kernel(C)
    nc = _kernel_cache[C]

    in_maps = _make_in_maps(xf, w1, b1, w2, weights, idxs, cnts, C)
    res = run_bass_kernel_spmd(nc, in_maps, core_ids=list(range(N_CORES)))

    out = np.zeros((T, DO), np.float32)
    for e in range(E):
        o_e = res.results[e]["o"].reshape(C, DO)
        out[idxs[e]] += o_e[: cnts[e]]
    out += weights @ b2  # b2 term, folded with the gate weights

    return out.reshape(B, S, DO), weights, expert_usage, top_i
